# revision 17
# baseline (speedup 1.0000x reference)
"""Trainium2 Bass kernel for nn_Attention_41841571398077.

Computation (per batch row b):
    p_imgs = imgs[b] @ W_v + b_v                                # [A, H]
    c      = h_att[b] @ W_ha + prev_h2[b] @ W_hv + b_ha + b_hv  # [H]
    att    = relu(p_imgs + c) @ W_f  (+ b_f, softmax-invariant) # [A]
    alpha  = softmax(att)                                       # [A]
    out[b] = alpha @ imgs[b]                                    # [DV]

Strategy: pure data parallel over batch across 8 NeuronCores (32 rows/core).
Host-side prep (layout/dtype only): imgs shard pre-transposed to [DV, 6272]
and cast to bf16 (weighted-sum stream) + fp8 e4m3 (projection stream,
W_v pre-scaled x32 to stay in e4m3 normal range; undone by scale=1/32 at
the ReLU eviction). Device work per core, pipelined over 4 pairs of 8 rows:
  * Plain contiguous HWDGE loads of the transposed streams (no on-device
    cast/transpose pass - xbar/SWDGE not needed at all).
  * Projection: fp8 DoubleRow matmuls (2 fp8 weights/cell, k-pairs on
    dim1 of [128, 16, cols] tiles), W_v chunk stationary, 392-col moving
    tiles, fp32 PSUM accumulation over 8 k-pair steps.
  * Bias(c per row) + ReLU fused into PSUM eviction on the scalar engine;
    c computed on-device from host-concatenated [h_att; prev_h2] bf16.
  * Scores: W_f chunk stationary [128,1], accumulated over 4 h-chunks.
  * Softmax without max-subtraction (scores O(1)-bounded), Exp+accum_out.
  * alpha broadcast across partitions via K=1 ones-matmul.
  * Weighted sum on DVE at 2x from the bf16 stream: rep-AP multiply +
    two halving adds + final 1x tensor_reduce (fp32).
  * Output assembled via PE transpose -> 512B-per-partition stores.
"""
import os
import sys

sys.path.insert(0, "/opt/trn_rl_repo")

import numpy as np
import ml_dtypes
from contextlib import ExitStack

import concourse.bass as bass
import concourse.tile as tile
from concourse import bacc, mybir
from concourse.bass_utils import run_bass_kernel_spmd

F32 = mybir.dt.float32
BF16 = mybir.dt.bfloat16
F8 = mybir.dt.float8e4
ACT = mybir.ActivationFunctionType
ALU = mybir.AluOpType
AX = mybir.AxisListType
DR = mybir.MatmulPerfMode.DoubleRow

B, A, DV, RNN, H = 256, 196, 2048, 1024, 512
NCORES = 8
BL = B // NCORES          # 32 rows/core
NP = 4                    # pairs of groups
RP = BL // NP             # 8 rows/pair
CP = RP * A               # 1568 cols/pair
HC = CP // 2              # 784 cols per half-load
U = 2 * A                 # 392 cols per 2-row unit
NC_DV = DV // 128         # 16 k-chunks
MH = H // 128             # 4 h-chunks
WV_SCALE = 32.0           # host pre-scale of W_v before e4m3 cast


def _install_ntff_shim():
    """Provide antenv.axon_hooks (NTFF profiling) if the image lacks it."""
    import contextlib
    import ctypes
    import types

    if "antenv.axon_hooks" in sys.modules:
        return
    so_path = "/opt/axon/libaxon_pjrt.so"
    try:
        lib = ctypes.CDLL(so_path)
    except OSError:
        return
    if not hasattr(lib, "axon_start_nrt_profile"):
        return
    lib.axon_start_nrt_profile.argtypes = [
        ctypes.POINTER(ctypes.c_int64),
        ctypes.c_size_t,
    ]
    lib.axon_start_nrt_profile.restype = ctypes.c_int64
    lib.axon_stop_nrt_profile.argtypes = [ctypes.c_char_p]
    lib.axon_stop_nrt_profile.restype = ctypes.c_int64

    @contextlib.contextmanager
    def _hook(output_dir, device_ids):
        import jax

        jax.devices()
        if device_ids:
            ids = (ctypes.c_int64 * len(device_ids))(*device_ids)
            rc = lib.axon_start_nrt_profile(ids, len(device_ids))
        else:
            rc = lib.axon_start_nrt_profile(None, 0)
        if rc != 0:
            raise RuntimeError(f"axon_start_nrt_profile rc={rc}")
        try:
            yield
        finally:
            n = lib.axon_stop_nrt_profile(str(output_dir).encode())
            if n <= 0:
                print(f"profile: {n} files written to {output_dir}", file=sys.stderr)

    mod = types.ModuleType("antenv.axon_hooks")
    mod.get_axon_ntff_profile_hook = lambda: _hook
    mod.set_axon_ntff_profile_hook = lambda h: None
    sys.modules["antenv.axon_hooks"] = mod


def build_kernel():
    nc = bacc.Bacc("TRN2", target_bir_lowering=False, debug=False)

    xt8 = nc.dram_tensor("xt8", [DV, BL * A], F8, kind="ExternalInput").ap()
    xtb = nc.dram_tensor("xtb", [DV, BL * A], BF16, kind="ExternalInput").ap()
    wv8 = nc.dram_tensor("wv8", [DV, H], F8, kind="ExternalInput").ap()
    wh = nc.dram_tensor("wh", [2 * RNN, H], BF16, kind="ExternalInput").ap()
    hsT = nc.dram_tensor("hsT", [2 * RNN, BL], BF16, kind="ExternalInput").ap()
    bsum = nc.dram_tensor("bsum", [H], F32, kind="ExternalInput").ap()
    wf = nc.dram_tensor("wf", [H], BF16, kind="ExternalInput").ap()
    out = nc.dram_tensor("out", [BL, DV], F32, kind="ExternalOutput").ap()

    xt8_r = xt8.rearrange("(c p) r -> p c r", p=128)
    xtb_r = xtb.rearrange("(c p) r -> p c r", p=128)

    with tile.TileContext(nc) as tc, ExitStack() as ctx:
        wpool = ctx.enter_context(tc.tile_pool(name="weights", bufs=1))
        xbpool = ctx.enter_context(tc.tile_pool(name="xtb", bufs=3))
        x8pool = ctx.enter_context(tc.tile_pool(name="xt8", bufs=6))
        rpool = ctx.enter_context(tc.tile_pool(name="relu", bufs=2))
        ppool = ctx.enter_context(tc.tile_pool(name="prod", bufs=1))
        spool = ctx.enter_context(tc.tile_pool(name="smax", bufs=3))
        apool = ctx.enter_context(tc.tile_pool(name="abc", bufs=3))
        opool = ctx.enter_context(tc.tile_pool(name="oacc", bufs=3))
        ps_proj = ctx.enter_context(tc.tile_pool(name="psp", bufs=4, space="PSUM"))
        ps_small = ctx.enter_context(tc.tile_pool(name="pss", bufs=4, space="PSUM"))

        # ---- persistent weights / constants ----
        hsT_sb = wpool.tile([128, NC_DV, BL], BF16)
        nc.gpsimd.dma_start(hsT_sb[:], hsT.rearrange("(c p) b -> p c b", p=128))
        bsum_sb = wpool.tile([128, MH], F32)
        nc.gpsimd.dma_start(bsum_sb[:], bsum.rearrange("(m p) -> p m", p=128))
        wf_sb = wpool.tile([128, MH], BF16)
        nc.gpsimd.dma_start(wf_sb[:], wf.rearrange("(m p) -> p m", p=128))

        from concourse.masks import make_identity
        ident_sb = wpool.tile([128, 128], F32)
        make_identity(nc, ident_sb[:])
        ones1_sb = wpool.tile([1, 1], F32)
        nc.vector.memset(ones1_sb[:], 1.0)

        # wh is only needed for the prologue c-matmul; borrow an xtb slot.
        wh_sb = xbpool.tile([128, NC_DV, H], BF16, tag="xtb", name="wh")
        nc.gpsimd.dma_start(wh_sb[:], wh.rearrange("(c p) h -> p c h", p=128))
        wv8_sb = wpool.tile([128, NC_DV, H], F8)
        nc.gpsimd.dma_start(wv8_sb[:], wv8.rearrange("(c p) h -> p c h", p=128))

        # c_sb[p, m, b] = (hs @ Wh)[b, m*128+p] + bsum[m*128+p]
        c_sb = wpool.tile([128, MH, BL], F32)
        for m in range(MH):
            psc = ps_proj.tile([128, BL], F32, tag="proj", name=f"psc{m}")
            for k in range(NC_DV):
                nc.tensor.matmul(
                    psc, wh_sb[:, k, m * 128 : (m + 1) * 128], hsT_sb[:, k, :],
                    start=(k == 0), stop=(k == NC_DV - 1),
                )
            nc.scalar.activation(
                c_sb[:, m, :], psc[:], ACT.Identity, bias=bsum_sb[:, m : m + 1]
            )

        # ---- main pipeline over 4-row groups ----
        # LDWEIGHTS is emitted per-matmul by the backend (no stationary
        # reuse), so small groups cost no extra PE time while halving the
        # pipeline fill and drain. Deferred stages keep every engine queue
        # free of cross-engine waits: the DVE weighted sum of group g is
        # emitted inside group g+1, and g's output transpose/store at the
        # end of group g+2 (when the weighted sum has long finished).
        NG = NP * 2               # 8 groups of 4 rows
        GC = CP // 2              # 784 cols/group
        GR = RP // 2              # 4 rows/group
        carry = {}

        def emit_ws(g):
            st = carry[g]
            exps, xb = st["exps"], st["xb"]
            o_acc = opool.tile([128, GR, NC_DV], F32, tag="oacc", name=f"oa_{g}")
            st["o_acc"] = o_acc
            for u in range(2):
                # broadcast (unnormalized) exp weights across partitions on
                # the idle GpSimd engine
                abc = apool.tile([128, U], BF16, tag="abc", name=f"abc_{g}_{u}")
                src = bass.AP(
                    tensor=exps.tensor, offset=exps.offset + 2 * u * A,
                    ap=[list(exps.ap[0]), [1, U]],
                )
                nc.gpsimd.partition_broadcast(abc[:], src)

                xv = bass.AP(
                    tensor=xb.tensor, offset=xb.offset + u * U,
                    ap=[list(xb.ap[0]), [A, 2], [GC, NC_DV], [1, A]],
                )
                av = bass.AP(
                    tensor=abc.tensor, offset=abc.offset,
                    ap=[list(abc.ap[0]), [A, 2], [0, NC_DV], [1, A]],
                )
                prod = ppool.tile([128, 2, NC_DV, A], BF16, tag="prod")
                nc.vector.tensor_mul(prod[:], xv, av)
                t1 = ppool.tile([128, 2, NC_DV, A // 2], BF16, tag="t1")
                nc.vector.tensor_add(
                    t1[:], prod[:, :, :, 0 : A // 2], prod[:, :, :, A // 2 : A]
                )
                t2 = ppool.tile([128, 2, NC_DV, A // 4], BF16, tag="t2")
                nc.vector.tensor_add(
                    t2[:], t1[:, :, :, 0 : A // 4], t1[:, :, :, A // 4 : A // 2]
                )
                t3 = ppool.tile([128, 2, NC_DV, 25], BF16, tag="t3")
                nc.vector.tensor_add(
                    t3[:, :, :, 0:24], t2[:, :, :, 0:24], t2[:, :, :, 25:49]
                )
                nc.vector.tensor_copy(t3[:, :, :, 24:25], t2[:, :, :, 24:25])
                nc.vector.tensor_reduce(
                    o_acc[:, 2 * u : 2 * u + 2, :], t3[:], axis=AX.X, op=ALU.add
                )

        def emit_out(g):
            st = carry.pop(g)
            o_acc, rec = st["o_acc"], st["rec"]
            # rec_bc[b*16+c] = 1/sums[b]: replicate x16 within partition 0
            # (scalar), then move to partitions via a K=1 matmul.
            rec16 = spool.tile([1, 64], F32, tag="rec16", name=f"r16_{g}")
            rsrc = bass.AP(
                tensor=rec.tensor, offset=rec.offset,
                ap=[list(rec.ap[0]), [1, GR], [0, NC_DV]],
            )
            nc.scalar.activation(rec16[:], rsrc, ACT.Copy)
            ps_rb = ps_small.tile([64, 1], F32, tag="small", name=f"prb_{g}")
            nc.tensor.matmul(ps_rb, rec16[:], ones1_sb[:], start=True, stop=True)
            rec_bc = spool.tile([64, 1], F32, tag="recbc", name=f"rbc_{g}")
            nc.scalar.activation(rec_bc[:], ps_rb[:], ACT.Copy)
            ps_t = ps_small.tile([64, 128], F32, tag="small", name=f"pt_{g}")
            nc.tensor.transpose(
                ps_t[:], o_acc.rearrange("p r c -> p (r c)"), ident_sb[:]
            )
            # normalization by 1/sum(exp) folded into the output eviction
            osb = opool.tile([64, 128], F32, tag="osb", name=f"osb_{g}")
            nc.scalar.activation(
                osb[:], ps_t[:], ACT.Copy, scale=rec_bc[:, 0:1]
            )
            nc.gpsimd.dma_start(
                out[g * GR : (g + 1) * GR].rearrange("b (c q) -> (b c) q", q=128),
                osb[:],
            )

        for g in range(NG):
            # fp8 stream in two k-halves so the first projection matmuls can
            # start when half has landed; the bf16 stream (needed only by the
            # weighted sum) is queued behind it on the same FIFO DMA ring.
            x8h = []
            for kh in range(2):
                t = x8pool.tile(
                    [128, NC_DV // 2, GC], F8, tag="xt8", name=f"x8_{g}_{kh}"
                )
                nc.sync.dma_start(
                    t[:],
                    xt8_r[:, kh * 8 : (kh + 1) * 8, g * GC : (g + 1) * GC],
                )
                x8h.append(t)

            relu_t = rpool.tile([128, MH, GC], BF16, tag="relu", name=f"relu_{g}")
            score_ps = [
                ps_small.tile([1, U], F32, tag="small", name=f"sc_{g}_{u}")
                for u in range(2)
            ]

            def emit_scores(m, score_ps=score_ps, relu_t=relu_t):
                for u in range(2):
                    nc.tensor.matmul(
                        score_ps[u], wf_sb[:, m : m + 1],
                        relu_t[:, m, u * U : (u + 1) * U],
                        start=(m == 0), stop=(m == MH - 1),
                    )

            for m in range(MH):
                psms = [
                    ps_proj.tile([128, U], F32, tag="proj", name=f"ps_{g}_{m}_{u}")
                    for u in range(2)
                ]
                for cp in range(NC_DV // 2):
                    cl = cp % 4
                    for u in range(2):
                        nc.tensor.matmul(
                            psms[u],
                            wv8_sb[:, 2 * cp : 2 * cp + 2, m * 128 : (m + 1) * 128],
                            x8h[cp // 4][:, 2 * cl : 2 * cl + 2, u * U : (u + 1) * U],
                            start=(cp == 0),
                            stop=(cp == NC_DV // 2 - 1),
                            perf_mode=DR,
                        )
                for u in range(2):
                    for b2 in range(2):
                        row = g * GR + u * 2 + b2
                        nc.scalar.activation(
                            relu_t[:, m, u * U + b2 * A : u * U + (b2 + 1) * A],
                            psms[u][:, b2 * A : (b2 + 1) * A],
                            ACT.Relu,
                            bias=c_sb[:, m, row : row + 1],
                            scale=1.0 / WV_SCALE,
                        )
                if m == 0:
                    xb = xbpool.tile(
                        [128, NC_DV, GC], BF16, tag="xtb", name=f"xb_{g}"
                    )
                    nc.sync.dma_start(
                        xb[:], xtb_r[:, :, g * GC : (g + 1) * GC]
                    )
                if m == 1 and g - 1 in carry:
                    emit_ws(g - 1)  # DVE work for group g-1 overlaps proj of g
                if m > 0:
                    emit_scores(m - 1)
            emit_scores(MH - 1)

            # softmax numerator (no max-sub: scores bounded ~[-6, 6] here);
            # exp weights stay unnormalized until the output eviction
            exps = spool.tile([1, GR, A], BF16, tag="exps", name=f"exps_{g}")
            sums = spool.tile([1, GR], F32, tag="sums", name=f"sums_{g}")
            for u in range(2):
                for b2 in range(2):
                    nc.scalar.activation(
                        exps[:, 2 * u + b2, :], score_ps[u][:, b2 * A : (b2 + 1) * A],
                        ACT.Exp, accum_out=sums[:, 2 * u + b2 : 2 * u + b2 + 1],
                    )
            rec = spool.tile([1, GR], F32, tag="rec", name=f"rec_{g}")
            nc.vector.reciprocal(rec[:], sums[:])
            carry[g] = {"exps": exps, "xb": xb, "rec": rec}

            if g - 2 in carry:
                emit_out(g - 2)

        emit_ws(NG - 1)
        emit_out(NG - 2)
        emit_out(NG - 1)

    nc.compile()
    return nc


_CACHE = {}


def _prep_core(imgs_c, f32=np.float32):
    """Transpose one core's imgs to [DV, BL*A] and cast to bf16 + e4m3."""
    xt = np.ascontiguousarray(imgs_c.reshape(BL * A, DV).T)
    return xt.astype(ml_dtypes.bfloat16), xt.astype(ml_dtypes.float8_e4m3)


def kernel(**inputs):
    inputs = {k: np.asarray(v) for k, v in inputs.items()}
    if "nc" not in _CACHE:
        _CACHE["nc"] = build_kernel()
    nc = _CACHE["nc"]

    wv8_np = (inputs["W_v"].astype(np.float32) * WV_SCALE).astype(
        ml_dtypes.float8_e4m3
    )
    wh_np = np.concatenate([inputs["W_ha"], inputs["W_hv"]], axis=0).astype(
        ml_dtypes.bfloat16
    )
    bsum_np = (inputs["b_v"] + inputs["b_ha"] + inputs["b_hv"]).astype(np.float32)
    wf_np = inputs["W_f"][:, 0].astype(ml_dtypes.bfloat16)
    hs = np.concatenate([inputs["h_att"], inputs["prev_h2"]], axis=1)  # [B, 2*RNN]

    in_maps = []
    for i in range(NCORES):
        s = slice(i * BL, (i + 1) * BL)
        xtb_np, xt8_np = _prep_core(inputs["imgs_features"][s])
        in_maps.append(
            {
                "xt8": xt8_np,
                "xtb": xtb_np,
                "wv8": wv8_np,
                "wh": wh_np,
                "hsT": np.ascontiguousarray(hs[s].T).astype(ml_dtypes.bfloat16),
                "bsum": bsum_np,
                "wf": wf_np,
            }
        )

    trace = bool(os.environ.get("BASS_KERNEL_TRACE"))
    if trace:
        _install_ntff_shim()
    res = run_bass_kernel_spmd(nc, in_maps, list(range(NCORES)), trace=trace)
    if trace:
        _CACHE["last_results"] = res
        print(f"HW exec time: {res.exec_time_ns} ns")
    return np.concatenate([res.results[i]["out"] for i in range(NCORES)], axis=0)


# revision 18
# speedup vs baseline: 1.0617x; 1.0617x over previous
"""Trainium2 Bass kernel for nn_Attention_41841571398077.

Computation (per batch row b):
    p_imgs = imgs[b] @ W_v + b_v                                # [A, H]
    c      = h_att[b] @ W_ha + prev_h2[b] @ W_hv + b_ha + b_hv  # [H]
    att    = relu(p_imgs + c) @ W_f  (+ b_f, softmax-invariant) # [A]
    alpha  = softmax(att)                                       # [A]
    out[b] = alpha @ imgs[b]                                    # [DV]

Strategy: pure data parallel over batch across 8 NeuronCores (32 rows/core).
Host-side prep (layout/dtype only): imgs shard pre-transposed to [DV, 6272]
and cast to bf16 (weighted-sum stream) + fp8 e4m3 (projection stream,
W_v pre-scaled x32 to stay in e4m3 normal range; undone by scale=1/32 at
the ReLU eviction). Device work per core, pipelined over 4 pairs of 8 rows:
  * Plain contiguous HWDGE loads of the transposed streams (no on-device
    cast/transpose pass - xbar/SWDGE not needed at all).
  * Projection: fp8 DoubleRow matmuls (2 fp8 weights/cell, k-pairs on
    dim1 of [128, 16, cols] tiles), W_v chunk stationary, 392-col moving
    tiles, fp32 PSUM accumulation over 8 k-pair steps.
  * Bias(c per row) + ReLU fused into PSUM eviction on the scalar engine;
    c computed on-device from host-concatenated [h_att; prev_h2] bf16.
  * Scores: W_f chunk stationary [128,1], accumulated over 4 h-chunks.
  * Softmax without max-subtraction (scores O(1)-bounded), Exp+accum_out.
  * alpha broadcast across partitions via K=1 ones-matmul.
  * Weighted sum on DVE at 2x from the bf16 stream: rep-AP multiply +
    two halving adds + final 1x tensor_reduce (fp32).
  * Output assembled via PE transpose -> 512B-per-partition stores.
"""
import os
import sys

sys.path.insert(0, "/opt/trn_rl_repo")

import numpy as np
import ml_dtypes
from contextlib import ExitStack

import concourse.bass as bass
import concourse.tile as tile
from concourse import bacc, mybir
from concourse.bass_utils import run_bass_kernel_spmd

F32 = mybir.dt.float32
BF16 = mybir.dt.bfloat16
F8 = mybir.dt.float8e4
ACT = mybir.ActivationFunctionType
ALU = mybir.AluOpType
AX = mybir.AxisListType
DR = mybir.MatmulPerfMode.DoubleRow

B, A, DV, RNN, H = 256, 196, 2048, 1024, 512
NCORES = 8
BL = B // NCORES          # 32 rows/core
NP = 4                    # pairs of groups
RP = BL // NP             # 8 rows/pair
CP = RP * A               # 1568 cols/pair
HC = CP // 2              # 784 cols per half-load
U = 2 * A                 # 392 cols per 2-row unit
NC_DV = DV // 128         # 16 k-chunks
MH = H // 128             # 4 h-chunks
WV_SCALE = 32.0           # host pre-scale of W_v before e4m3 cast


def _install_ntff_shim():
    """Provide antenv.axon_hooks (NTFF profiling) if the image lacks it."""
    import contextlib
    import ctypes
    import types

    if "antenv.axon_hooks" in sys.modules:
        return
    so_path = "/opt/axon/libaxon_pjrt.so"
    try:
        lib = ctypes.CDLL(so_path)
    except OSError:
        return
    if not hasattr(lib, "axon_start_nrt_profile"):
        return
    lib.axon_start_nrt_profile.argtypes = [
        ctypes.POINTER(ctypes.c_int64),
        ctypes.c_size_t,
    ]
    lib.axon_start_nrt_profile.restype = ctypes.c_int64
    lib.axon_stop_nrt_profile.argtypes = [ctypes.c_char_p]
    lib.axon_stop_nrt_profile.restype = ctypes.c_int64

    @contextlib.contextmanager
    def _hook(output_dir, device_ids):
        import jax

        jax.devices()
        if device_ids:
            ids = (ctypes.c_int64 * len(device_ids))(*device_ids)
            rc = lib.axon_start_nrt_profile(ids, len(device_ids))
        else:
            rc = lib.axon_start_nrt_profile(None, 0)
        if rc != 0:
            raise RuntimeError(f"axon_start_nrt_profile rc={rc}")
        try:
            yield
        finally:
            n = lib.axon_stop_nrt_profile(str(output_dir).encode())
            if n <= 0:
                print(f"profile: {n} files written to {output_dir}", file=sys.stderr)

    mod = types.ModuleType("antenv.axon_hooks")
    mod.get_axon_ntff_profile_hook = lambda: _hook
    mod.set_axon_ntff_profile_hook = lambda h: None
    sys.modules["antenv.axon_hooks"] = mod


def build_kernel():
    nc = bacc.Bacc("TRN2", target_bir_lowering=False, debug=False)

    xt8 = nc.dram_tensor("xt8", [DV, BL * A], F8, kind="ExternalInput").ap()
    xtb = nc.dram_tensor("xtb", [DV, BL * A], BF16, kind="ExternalInput").ap()
    wv8 = nc.dram_tensor("wv8", [DV, H], F8, kind="ExternalInput").ap()
    wh = nc.dram_tensor("wh", [2 * RNN, H], BF16, kind="ExternalInput").ap()
    hsT = nc.dram_tensor("hsT", [2 * RNN, BL], BF16, kind="ExternalInput").ap()
    bsum = nc.dram_tensor("bsum", [H], F32, kind="ExternalInput").ap()
    wf = nc.dram_tensor("wf", [H], BF16, kind="ExternalInput").ap()
    out = nc.dram_tensor("out", [BL, DV], F32, kind="ExternalOutput").ap()

    xt8_r = xt8.rearrange("(c p) r -> p c r", p=128)
    xtb_r = xtb.rearrange("(c p) r -> p c r", p=128)

    with tile.TileContext(nc) as tc, ExitStack() as ctx:
        wpool = ctx.enter_context(tc.tile_pool(name="weights", bufs=1))
        xbpool = ctx.enter_context(tc.tile_pool(name="xtb", bufs=3))
        x8pool = ctx.enter_context(tc.tile_pool(name="xt8", bufs=6))
        rpool = ctx.enter_context(tc.tile_pool(name="relu", bufs=2))
        ppool = ctx.enter_context(tc.tile_pool(name="prod", bufs=1))
        spool = ctx.enter_context(tc.tile_pool(name="smax", bufs=3))
        apool = ctx.enter_context(tc.tile_pool(name="abc", bufs=3))
        opool = ctx.enter_context(tc.tile_pool(name="oacc", bufs=3))
        ps_proj = ctx.enter_context(tc.tile_pool(name="psp", bufs=4, space="PSUM"))
        ps_small = ctx.enter_context(tc.tile_pool(name="pss", bufs=4, space="PSUM"))

        # ---- persistent weights / constants ----
        hsT_sb = wpool.tile([128, NC_DV, BL], BF16)
        nc.gpsimd.dma_start(hsT_sb[:], hsT.rearrange("(c p) b -> p c b", p=128))
        bsum_sb = wpool.tile([128, MH], F32)
        nc.gpsimd.dma_start(bsum_sb[:], bsum.rearrange("(m p) -> p m", p=128))
        wf_sb = wpool.tile([128, MH], BF16)
        nc.gpsimd.dma_start(wf_sb[:], wf.rearrange("(m p) -> p m", p=128))

        from concourse.masks import make_identity
        ident_sb = wpool.tile([128, 128], F32)
        make_identity(nc, ident_sb[:])
        ones1_sb = wpool.tile([1, 1], F32)
        nc.vector.memset(ones1_sb[:], 1.0)

        # wh is only needed for the prologue c-matmul; borrow an xtb slot.
        # Loaded in 4 m-chunks so the first c-block (needed by the first
        # ReLU eviction) is not gated on the full 2MB transfer.
        wh_sb = xbpool.tile([128, NC_DV, H], BF16, tag="xtb", name="wh")
        wh_r = wh.rearrange("(c p) h -> p c h", p=128)
        for m in range(MH):
            nc.gpsimd.dma_start(
                wh_sb[:, :, m * 128 : (m + 1) * 128],
                wh_r[:, :, m * 128 : (m + 1) * 128],
            )
        wv8_sb = wpool.tile([128, NC_DV, H], F8)
        nc.gpsimd.dma_start(wv8_sb[:], wv8.rearrange("(c p) h -> p c h", p=128))

        # c_sb[p, m, b] = (hs @ Wh)[b, m*128+p] + bsum[m*128+p]
        # (matmuls emitted inside group 0's m-loop, interleaved after each
        # projection block, so the PE FIFO never waits on the wh transfer)
        c_sb = wpool.tile([128, MH, BL], F32)

        def emit_c_block(m):
            psc = ps_proj.tile([128, BL], F32, tag="proj", name=f"psc{m}")
            for k in range(NC_DV):
                nc.tensor.matmul(
                    psc, wh_sb[:, k, m * 128 : (m + 1) * 128], hsT_sb[:, k, :],
                    start=(k == 0), stop=(k == NC_DV - 1),
                )
            nc.scalar.activation(
                c_sb[:, m, :], psc[:], ACT.Identity, bias=bsum_sb[:, m : m + 1]
            )

        # ---- main pipeline over 4-row groups ----
        # LDWEIGHTS is emitted per-matmul by the backend (no stationary
        # reuse), so small groups cost no extra PE time while halving the
        # pipeline fill and drain. Deferred stages keep every engine queue
        # free of cross-engine waits: the DVE weighted sum of group g is
        # emitted inside group g+1, and g's output transpose/store at the
        # end of group g+2 (when the weighted sum has long finished).
        NG = NP * 2               # 8 groups of 4 rows
        GC = CP // 2              # 784 cols/group
        GR = RP // 2              # 4 rows/group
        carry = {}

        def emit_ws(g):
            st = carry[g]
            exps, xb = st["exps"], st["xb"]
            o_acc = opool.tile([128, GR, NC_DV], F32, tag="oacc", name=f"oa_{g}")
            st["o_acc"] = o_acc
            for u in range(2):
                # broadcast (unnormalized) exp weights across partitions on
                # the idle GpSimd engine
                abc = apool.tile([128, U], BF16, tag="abc", name=f"abc_{g}_{u}")
                src = bass.AP(
                    tensor=exps.tensor, offset=exps.offset + 2 * u * A,
                    ap=[list(exps.ap[0]), [1, U]],
                )
                nc.gpsimd.partition_broadcast(abc[:], src)

                xv = bass.AP(
                    tensor=xb.tensor, offset=xb.offset + u * U,
                    ap=[list(xb.ap[0]), [A, 2], [GC, NC_DV], [1, A]],
                )
                av = bass.AP(
                    tensor=abc.tensor, offset=abc.offset,
                    ap=[list(abc.ap[0]), [A, 2], [0, NC_DV], [1, A]],
                )
                prod = ppool.tile([128, 2, NC_DV, A], BF16, tag="prod")
                nc.vector.tensor_mul(prod[:], xv, av)
                t1 = ppool.tile([128, 2, NC_DV, A // 2], BF16, tag="t1")
                nc.vector.tensor_add(
                    t1[:], prod[:, :, :, 0 : A // 2], prod[:, :, :, A // 2 : A]
                )
                t2 = ppool.tile([128, 2, NC_DV, A // 4], BF16, tag="t2")
                nc.vector.tensor_add(
                    t2[:], t1[:, :, :, 0 : A // 4], t1[:, :, :, A // 4 : A // 2]
                )
                nc.vector.tensor_reduce(
                    o_acc[:, 2 * u : 2 * u + 2, :], t2[:], axis=AX.X, op=ALU.add
                )

        def emit_out(g):
            st = carry.pop(g)
            o_acc, rec = st["o_acc"], st["rec"]
            # rec_bc[b*16+c] = 1/sums[b]: replicate x16 within partition 0
            # (scalar), then move to partitions via a K=1 matmul.
            rec16 = spool.tile([1, 64], F32, tag="rec16", name=f"r16_{g}")
            rsrc = bass.AP(
                tensor=rec.tensor, offset=rec.offset,
                ap=[list(rec.ap[0]), [1, GR], [0, NC_DV]],
            )
            nc.scalar.activation(rec16[:], rsrc, ACT.Copy)
            ps_rb = ps_small.tile([64, 1], F32, tag="small", name=f"prb_{g}")
            nc.tensor.matmul(ps_rb, rec16[:], ones1_sb[:], start=True, stop=True)
            rec_bc = spool.tile([64, 1], F32, tag="recbc", name=f"rbc_{g}")
            nc.scalar.activation(rec_bc[:], ps_rb[:], ACT.Copy)
            ps_t = ps_small.tile([64, 128], F32, tag="small", name=f"pt_{g}")
            nc.tensor.transpose(
                ps_t[:], o_acc.rearrange("p r c -> p (r c)"), ident_sb[:]
            )
            # normalization by 1/sum(exp) folded into the output eviction
            osb = opool.tile([64, 128], F32, tag="osb", name=f"osb_{g}")
            nc.scalar.activation(
                osb[:], ps_t[:], ACT.Copy, scale=rec_bc[:, 0:1]
            )
            nc.scalar.dma_start(
                out[g * GR : (g + 1) * GR].rearrange("b (c q) -> (b c) q", q=128),
                osb[:],
            )

        for g in range(NG):
            # fp8 stream in two k-halves so the first projection matmuls can
            # start when half has landed; the bf16 stream (needed only by the
            # weighted sum) is queued behind it on the same FIFO DMA ring.
            x8h = []
            for kh in range(2):
                t = x8pool.tile(
                    [128, NC_DV // 2, GC], F8, tag="xt8", name=f"x8_{g}_{kh}"
                )
                nc.sync.dma_start(
                    t[:],
                    xt8_r[:, kh * 8 : (kh + 1) * 8, g * GC : (g + 1) * GC],
                )
                x8h.append(t)

            relu_t = rpool.tile([128, MH, GC], BF16, tag="relu", name=f"relu_{g}")
            score_ps = [
                ps_small.tile([1, U], F32, tag="small", name=f"sc_{g}_{u}")
                for u in range(2)
            ]

            def emit_scores(m, score_ps=score_ps, relu_t=relu_t):
                for u in range(2):
                    nc.tensor.matmul(
                        score_ps[u], wf_sb[:, m : m + 1],
                        relu_t[:, m, u * U : (u + 1) * U],
                        start=(m == 0), stop=(m == MH - 1),
                    )

            for m in range(MH):
                psms = [
                    ps_proj.tile([128, U], F32, tag="proj", name=f"ps_{g}_{m}_{u}")
                    for u in range(2)
                ]
                for cp in range(NC_DV // 2):
                    cl = cp % 4
                    for u in range(2):
                        nc.tensor.matmul(
                            psms[u],
                            wv8_sb[:, 2 * cp : 2 * cp + 2, m * 128 : (m + 1) * 128],
                            x8h[cp // 4][:, 2 * cl : 2 * cl + 2, u * U : (u + 1) * U],
                            start=(cp == 0),
                            stop=(cp == NC_DV // 2 - 1),
                            perf_mode=DR,
                        )
                if g == 0:
                    emit_c_block(m)
                for u in range(2):
                    for b2 in range(2):
                        row = g * GR + u * 2 + b2
                        nc.scalar.activation(
                            relu_t[:, m, u * U + b2 * A : u * U + (b2 + 1) * A],
                            psms[u][:, b2 * A : (b2 + 1) * A],
                            ACT.Relu,
                            bias=c_sb[:, m, row : row + 1],
                            scale=1.0 / WV_SCALE,
                        )
                if m == 0:
                    xb = xbpool.tile(
                        [128, NC_DV, GC], BF16, tag="xtb", name=f"xb_{g}"
                    )
                    nc.sync.dma_start(
                        xb[:], xtb_r[:, :, g * GC : (g + 1) * GC]
                    )
                if m == 1 and g - 1 in carry:
                    emit_ws(g - 1)  # DVE work for group g-1 overlaps proj of g
                if m > 0:
                    emit_scores(m - 1)
            emit_scores(MH - 1)

            # softmax numerator (no max-sub: scores bounded ~[-6, 6] here);
            # exp weights stay unnormalized until the output eviction
            exps = spool.tile([1, GR, A], BF16, tag="exps", name=f"exps_{g}")
            sums = spool.tile([1, GR], F32, tag="sums", name=f"sums_{g}")
            for u in range(2):
                for b2 in range(2):
                    nc.scalar.activation(
                        exps[:, 2 * u + b2, :], score_ps[u][:, b2 * A : (b2 + 1) * A],
                        ACT.Exp, accum_out=sums[:, 2 * u + b2 : 2 * u + b2 + 1],
                    )
            rec = spool.tile([1, GR], F32, tag="rec", name=f"rec_{g}")
            nc.vector.reciprocal(rec[:], sums[:])
            carry[g] = {"exps": exps, "xb": xb, "rec": rec}

            if g - 2 in carry:
                emit_out(g - 2)

        emit_ws(NG - 1)
        emit_out(NG - 2)
        emit_out(NG - 1)

    nc.compile()
    return nc


_CACHE = {}


def _prep_core(imgs_c, f32=np.float32):
    """Transpose one core's imgs to [DV, BL*A] and cast to bf16 + e4m3."""
    xt = np.ascontiguousarray(imgs_c.reshape(BL * A, DV).T)
    return xt.astype(ml_dtypes.bfloat16), xt.astype(ml_dtypes.float8_e4m3)


def kernel(**inputs):
    inputs = {k: np.asarray(v) for k, v in inputs.items()}
    if "nc" not in _CACHE:
        _CACHE["nc"] = build_kernel()
    nc = _CACHE["nc"]

    wv8_np = (inputs["W_v"].astype(np.float32) * WV_SCALE).astype(
        ml_dtypes.float8_e4m3
    )
    wh_np = np.concatenate([inputs["W_ha"], inputs["W_hv"]], axis=0).astype(
        ml_dtypes.bfloat16
    )
    bsum_np = (inputs["b_v"] + inputs["b_ha"] + inputs["b_hv"]).astype(np.float32)
    wf_np = inputs["W_f"][:, 0].astype(ml_dtypes.bfloat16)
    hs = np.concatenate([inputs["h_att"], inputs["prev_h2"]], axis=1)  # [B, 2*RNN]

    in_maps = []
    for i in range(NCORES):
        s = slice(i * BL, (i + 1) * BL)
        xtb_np, xt8_np = _prep_core(inputs["imgs_features"][s])
        in_maps.append(
            {
                "xt8": xt8_np,
                "xtb": xtb_np,
                "wv8": wv8_np,
                "wh": wh_np,
                "hsT": np.ascontiguousarray(hs[s].T).astype(ml_dtypes.bfloat16),
                "bsum": bsum_np,
                "wf": wf_np,
            }
        )

    trace = bool(os.environ.get("BASS_KERNEL_TRACE"))
    if trace:
        _install_ntff_shim()
    res = run_bass_kernel_spmd(nc, in_maps, list(range(NCORES)), trace=trace)
    if trace:
        _CACHE["last_results"] = res
        print(f"HW exec time: {res.exec_time_ns} ns")
    return np.concatenate([res.results[i]["out"] for i in range(NCORES)], axis=0)


# revision 19
# speedup vs baseline: 1.1180x; 1.0531x over previous
"""Trainium2 Bass kernel for nn_Attention_41841571398077.

Computation (per batch row b):
    p_imgs = imgs[b] @ W_v + b_v                                # [A, H]
    c      = h_att[b] @ W_ha + prev_h2[b] @ W_hv + b_ha + b_hv  # [H]
    att    = relu(p_imgs + c) @ W_f  (+ b_f, softmax-invariant) # [A]
    alpha  = softmax(att)                                       # [A]
    out[b] = alpha @ imgs[b]                                    # [DV]

Strategy: pure data parallel over batch across 8 NeuronCores (32 rows/core).
Host-side prep (layout/dtype only): imgs shard pre-transposed to [DV, 6272]
and cast to bf16 (weighted-sum stream) + fp8 e4m3 (projection stream,
W_v pre-scaled x32 to stay in e4m3 normal range; undone by scale=1/32 at
the ReLU eviction). Device work per core, pipelined over 4 pairs of 8 rows:
  * Plain contiguous HWDGE loads of the transposed streams (no on-device
    cast/transpose pass - xbar/SWDGE not needed at all).
  * Projection: fp8 DoubleRow matmuls (2 fp8 weights/cell, k-pairs on
    dim1 of [128, 16, cols] tiles), W_v chunk stationary, 392-col moving
    tiles, fp32 PSUM accumulation over 8 k-pair steps.
  * Bias(c per row) + ReLU fused into PSUM eviction on the scalar engine;
    c computed on-device from host-concatenated [h_att; prev_h2] bf16.
  * Scores: W_f chunk stationary [128,1], accumulated over 4 h-chunks.
  * Softmax without max-subtraction (scores O(1)-bounded), Exp+accum_out.
  * alpha broadcast across partitions via K=1 ones-matmul.
  * Weighted sum on DVE at 2x from the bf16 stream: rep-AP multiply +
    two halving adds + final 1x tensor_reduce (fp32).
  * Output assembled via PE transpose -> 512B-per-partition stores.
"""
import os
import sys

sys.path.insert(0, "/opt/trn_rl_repo")

import numpy as np
import ml_dtypes
from contextlib import ExitStack

import concourse.bass as bass
import concourse.tile as tile
from concourse import bacc, mybir
from concourse.bass_utils import run_bass_kernel_spmd

F32 = mybir.dt.float32
BF16 = mybir.dt.bfloat16
F8 = mybir.dt.float8e4
ACT = mybir.ActivationFunctionType
ALU = mybir.AluOpType
AX = mybir.AxisListType
DR = mybir.MatmulPerfMode.DoubleRow

B, A, DV, RNN, H = 256, 196, 2048, 1024, 512
NCORES = 8
BL = B // NCORES          # 32 rows/core
NP = 4                    # pairs of groups
RP = BL // NP             # 8 rows/pair
CP = RP * A               # 1568 cols/pair
HC = CP // 2              # 784 cols per half-load
U = 2 * A                 # 392 cols per 2-row unit
NC_DV = DV // 128         # 16 k-chunks
MH = H // 128             # 4 h-chunks
WV_SCALE = 32.0           # host pre-scale of W_v before e4m3 cast


def _install_ntff_shim():
    """Provide antenv.axon_hooks (NTFF profiling) if the image lacks it."""
    import contextlib
    import ctypes
    import types

    if "antenv.axon_hooks" in sys.modules:
        return
    so_path = "/opt/axon/libaxon_pjrt.so"
    try:
        lib = ctypes.CDLL(so_path)
    except OSError:
        return
    if not hasattr(lib, "axon_start_nrt_profile"):
        return
    lib.axon_start_nrt_profile.argtypes = [
        ctypes.POINTER(ctypes.c_int64),
        ctypes.c_size_t,
    ]
    lib.axon_start_nrt_profile.restype = ctypes.c_int64
    lib.axon_stop_nrt_profile.argtypes = [ctypes.c_char_p]
    lib.axon_stop_nrt_profile.restype = ctypes.c_int64

    @contextlib.contextmanager
    def _hook(output_dir, device_ids):
        import jax

        jax.devices()
        if device_ids:
            ids = (ctypes.c_int64 * len(device_ids))(*device_ids)
            rc = lib.axon_start_nrt_profile(ids, len(device_ids))
        else:
            rc = lib.axon_start_nrt_profile(None, 0)
        if rc != 0:
            raise RuntimeError(f"axon_start_nrt_profile rc={rc}")
        try:
            yield
        finally:
            n = lib.axon_stop_nrt_profile(str(output_dir).encode())
            if n <= 0:
                print(f"profile: {n} files written to {output_dir}", file=sys.stderr)

    mod = types.ModuleType("antenv.axon_hooks")
    mod.get_axon_ntff_profile_hook = lambda: _hook
    mod.set_axon_ntff_profile_hook = lambda h: None
    sys.modules["antenv.axon_hooks"] = mod


def build_kernel():
    nc = bacc.Bacc("TRN2", target_bir_lowering=False, debug=False)

    xt8 = nc.dram_tensor("xt8", [DV, BL * A], F8, kind="ExternalInput").ap()
    xtb = nc.dram_tensor("xtb", [DV, BL * A], BF16, kind="ExternalInput").ap()
    wv8 = nc.dram_tensor("wv8", [DV, H], F8, kind="ExternalInput").ap()
    wh = nc.dram_tensor("wh", [2 * RNN, H], BF16, kind="ExternalInput").ap()
    hsT = nc.dram_tensor("hsT", [2 * RNN, BL], BF16, kind="ExternalInput").ap()
    bsum = nc.dram_tensor("bsum", [H], F32, kind="ExternalInput").ap()
    wf = nc.dram_tensor("wf", [H], BF16, kind="ExternalInput").ap()
    out = nc.dram_tensor("out", [BL, DV], F32, kind="ExternalOutput").ap()

    xt8_r = xt8.rearrange("(c p) r -> p c r", p=128)
    xtb_r = xtb.rearrange("(c p) r -> p c r", p=128)

    with tile.TileContext(nc) as tc, ExitStack() as ctx:
        wpool = ctx.enter_context(tc.tile_pool(name="weights", bufs=1))
        xbpool = ctx.enter_context(tc.tile_pool(name="xtb", bufs=3))
        x8pool = ctx.enter_context(tc.tile_pool(name="xt8", bufs=6))
        rpool = ctx.enter_context(tc.tile_pool(name="relu", bufs=2))
        ppool = ctx.enter_context(tc.tile_pool(name="prod", bufs=1))
        spool = ctx.enter_context(tc.tile_pool(name="smax", bufs=3))
        apool = ctx.enter_context(tc.tile_pool(name="abc", bufs=3))
        opool = ctx.enter_context(tc.tile_pool(name="oacc", bufs=3))
        ps_proj = ctx.enter_context(tc.tile_pool(name="psp", bufs=4, space="PSUM"))
        ps_small = ctx.enter_context(tc.tile_pool(name="pss", bufs=4, space="PSUM"))

        # ---- persistent weights / constants ----
        # wv8 first: the first projection matmul gates on it
        wv8_sb = wpool.tile([128, NC_DV, H], F8)
        nc.gpsimd.dma_start(wv8_sb[:], wv8.rearrange("(c p) h -> p c h", p=128))
        hsT_sb = wpool.tile([128, NC_DV, BL], BF16)
        nc.gpsimd.dma_start(hsT_sb[:], hsT.rearrange("(c p) b -> p c b", p=128))
        bsum_sb = wpool.tile([128, MH], F32)
        nc.gpsimd.dma_start(bsum_sb[:], bsum.rearrange("(m p) -> p m", p=128))
        wf_sb = wpool.tile([128, MH], BF16)
        nc.gpsimd.dma_start(wf_sb[:], wf.rearrange("(m p) -> p m", p=128))

        from concourse.masks import make_identity
        ident_sb = wpool.tile([128, 128], F32)
        make_identity(nc, ident_sb[:])
        ones1_sb = wpool.tile([1, 1], F32)
        nc.vector.memset(ones1_sb[:], 1.0)

        # wh is only needed for the prologue c-matmul; borrow an xtb slot.
        # Loaded in 4 m-chunks so the first c-block (needed by the first
        # ReLU eviction) is not gated on the full 2MB transfer.
        wh_sb = xbpool.tile([128, NC_DV, H], BF16, tag="xtb", name="wh")
        wh_r = wh.rearrange("(c p) h -> p c h", p=128)
        for m in range(MH):
            nc.gpsimd.dma_start(
                wh_sb[:, :, m * 128 : (m + 1) * 128],
                wh_r[:, :, m * 128 : (m + 1) * 128],
            )

        # c_sb[p, m, b] = (hs @ Wh)[b, m*128+p] + bsum[m*128+p]
        # (matmuls emitted inside group 0's m-loop, interleaved after each
        # projection block, so the PE FIFO never waits on the wh transfer)
        c_sb = wpool.tile([128, MH, BL], F32)

        def emit_c_block(m):
            psc = ps_proj.tile([128, BL], F32, tag="proj", name=f"psc{m}")
            for k in range(NC_DV):
                nc.tensor.matmul(
                    psc, wh_sb[:, k, m * 128 : (m + 1) * 128], hsT_sb[:, k, :],
                    start=(k == 0), stop=(k == NC_DV - 1),
                )
            nc.scalar.activation(
                c_sb[:, m, :], psc[:], ACT.Identity, bias=bsum_sb[:, m : m + 1]
            )

        # ---- main pipeline over 4-row groups ----
        # LDWEIGHTS is emitted per-matmul by the backend (no stationary
        # reuse), so small groups cost no extra PE time while halving the
        # pipeline fill and drain. Deferred stages keep every engine queue
        # free of cross-engine waits: the DVE weighted sum of group g is
        # emitted inside group g+1, and g's output transpose/store at the
        # end of group g+2 (when the weighted sum has long finished).
        NG = NP * 2               # 8 groups of 4 rows
        GC = CP // 2              # 784 cols/group
        GR = RP // 2              # 4 rows/group
        carry = {}

        def emit_ws(g):
            st = carry[g]
            exps, xb = st["exps"], st["xb"]
            o_acc = opool.tile([128, GR, NC_DV], F32, tag="oacc", name=f"oa_{g}")
            st["o_acc"] = o_acc
            for u in range(2):
                # broadcast (unnormalized) exp weights across partitions on
                # the idle GpSimd engine
                abc = apool.tile([128, U], BF16, tag="abc", name=f"abc_{g}_{u}")
                src = bass.AP(
                    tensor=exps.tensor, offset=exps.offset + 2 * u * A,
                    ap=[list(exps.ap[0]), [1, U]],
                )
                nc.gpsimd.partition_broadcast(abc[:], src)

                xv = bass.AP(
                    tensor=xb.tensor, offset=xb.offset + u * U,
                    ap=[list(xb.ap[0]), [A, 2], [GC, NC_DV], [1, A]],
                )
                av = bass.AP(
                    tensor=abc.tensor, offset=abc.offset,
                    ap=[list(abc.ap[0]), [A, 2], [0, NC_DV], [1, A]],
                )
                prod = ppool.tile([128, 2, NC_DV, A], BF16, tag="prod")
                nc.vector.tensor_mul(prod[:], xv, av)
                t1 = ppool.tile([128, 2, NC_DV, A // 2], BF16, tag="t1")
                nc.vector.tensor_add(
                    t1[:], prod[:, :, :, 0 : A // 2], prod[:, :, :, A // 2 : A]
                )
                t2 = ppool.tile([128, 2, NC_DV, A // 4], BF16, tag="t2")
                nc.vector.tensor_add(
                    t2[:], t1[:, :, :, 0 : A // 4], t1[:, :, :, A // 4 : A // 2]
                )
                nc.vector.tensor_reduce(
                    o_acc[:, 2 * u : 2 * u + 2, :], t2[:], axis=AX.X, op=ALU.add
                )

        def emit_out(g):
            st = carry.pop(g)
            o_acc, rec = st["o_acc"], st["rec"]
            # rec_bc[b*16+c] = 1/sums[b]: replicate x16 within partition 0
            # (scalar), then move to partitions via a K=1 matmul.
            rec16 = spool.tile([1, 64], F32, tag="rec16", name=f"r16_{g}")
            rsrc = bass.AP(
                tensor=rec.tensor, offset=rec.offset,
                ap=[list(rec.ap[0]), [1, GR], [0, NC_DV]],
            )
            nc.scalar.activation(rec16[:], rsrc, ACT.Copy)
            ps_rb = ps_small.tile([64, 1], F32, tag="small", name=f"prb_{g}")
            nc.tensor.matmul(ps_rb, rec16[:], ones1_sb[:], start=True, stop=True)
            rec_bc = spool.tile([64, 1], F32, tag="recbc", name=f"rbc_{g}")
            nc.scalar.activation(rec_bc[:], ps_rb[:], ACT.Copy)
            ps_t = ps_small.tile([64, 128], F32, tag="small", name=f"pt_{g}")
            nc.tensor.transpose(
                ps_t[:], o_acc.rearrange("p r c -> p (r c)"), ident_sb[:]
            )
            # normalization by 1/sum(exp) folded into the output eviction
            osb = opool.tile([64, 128], F32, tag="osb", name=f"osb_{g}")
            nc.scalar.activation(
                osb[:], ps_t[:], ACT.Copy, scale=rec_bc[:, 0:1]
            )
            nc.scalar.dma_start(
                out[g * GR : (g + 1) * GR].rearrange("b (c q) -> (b c) q", q=128),
                osb[:],
            )

        for g in range(NG):
            # fp8 stream in two k-halves so the first projection matmuls can
            # start when half has landed; the bf16 stream (needed only by the
            # weighted sum) is queued behind it on the same FIFO DMA ring.
            x8h = []
            for kh in range(2):
                t = x8pool.tile(
                    [128, NC_DV // 2, GC], F8, tag="xt8", name=f"x8_{g}_{kh}"
                )
                nc.sync.dma_start(
                    t[:],
                    xt8_r[:, kh * 8 : (kh + 1) * 8, g * GC : (g + 1) * GC],
                )
                x8h.append(t)

            relu_t = rpool.tile([128, MH, GC], BF16, tag="relu", name=f"relu_{g}")
            score_ps = [
                ps_small.tile([1, U], F32, tag="small", name=f"sc_{g}_{u}")
                for u in range(2)
            ]

            def emit_scores(m, score_ps=score_ps, relu_t=relu_t):
                for u in range(2):
                    nc.tensor.matmul(
                        score_ps[u], wf_sb[:, m : m + 1],
                        relu_t[:, m, u * U : (u + 1) * U],
                        start=(m == 0), stop=(m == MH - 1),
                    )

            for m in range(MH):
                psms = [
                    ps_proj.tile([128, U], F32, tag="proj", name=f"ps_{g}_{m}_{u}")
                    for u in range(2)
                ]
                for cp in range(NC_DV // 2):
                    cl = cp % 4
                    for u in range(2):
                        nc.tensor.matmul(
                            psms[u],
                            wv8_sb[:, 2 * cp : 2 * cp + 2, m * 128 : (m + 1) * 128],
                            x8h[cp // 4][:, 2 * cl : 2 * cl + 2, u * U : (u + 1) * U],
                            start=(cp == 0),
                            stop=(cp == NC_DV // 2 - 1),
                            perf_mode=DR,
                        )
                if g == 0:
                    emit_c_block(m)
                for u in range(2):
                    for b2 in range(2):
                        row = g * GR + u * 2 + b2
                        nc.scalar.activation(
                            relu_t[:, m, u * U + b2 * A : u * U + (b2 + 1) * A],
                            psms[u][:, b2 * A : (b2 + 1) * A],
                            ACT.Relu,
                            bias=c_sb[:, m, row : row + 1],
                            scale=1.0 / WV_SCALE,
                        )
                if m == 0:
                    xb = xbpool.tile(
                        [128, NC_DV, GC], BF16, tag="xtb", name=f"xb_{g}"
                    )
                    nc.sync.dma_start(
                        xb[:], xtb_r[:, :, g * GC : (g + 1) * GC]
                    )
                if m == 1 and g - 1 in carry:
                    emit_ws(g - 1)  # DVE work for group g-1 overlaps proj of g
                if m > 0:
                    emit_scores(m - 1)
            emit_scores(MH - 1)

            # softmax numerator (no max-sub: scores bounded ~[-6, 6] here);
            # exp weights stay unnormalized until the output eviction
            exps = spool.tile([1, GR, A], BF16, tag="exps", name=f"exps_{g}")
            sums = spool.tile([1, GR], F32, tag="sums", name=f"sums_{g}")
            for u in range(2):
                for b2 in range(2):
                    nc.scalar.activation(
                        exps[:, 2 * u + b2, :], score_ps[u][:, b2 * A : (b2 + 1) * A],
                        ACT.Exp, accum_out=sums[:, 2 * u + b2 : 2 * u + b2 + 1],
                    )
            rec = spool.tile([1, GR], F32, tag="rec", name=f"rec_{g}")
            nc.vector.reciprocal(rec[:], sums[:])
            carry[g] = {"exps": exps, "xb": xb, "rec": rec}

            if g - 2 in carry:
                emit_out(g - 2)

        emit_ws(NG - 1)
        emit_out(NG - 2)
        emit_out(NG - 1)

    nc.compile()
    return nc


_CACHE = {}


def _prep_core(imgs_c, f32=np.float32):
    """Transpose one core's imgs to [DV, BL*A] and cast to bf16 + e4m3."""
    xt = np.ascontiguousarray(imgs_c.reshape(BL * A, DV).T)
    return xt.astype(ml_dtypes.bfloat16), xt.astype(ml_dtypes.float8_e4m3)


def kernel(**inputs):
    inputs = {k: np.asarray(v) for k, v in inputs.items()}
    if "nc" not in _CACHE:
        _CACHE["nc"] = build_kernel()
    nc = _CACHE["nc"]

    wv8_np = (inputs["W_v"].astype(np.float32) * WV_SCALE).astype(
        ml_dtypes.float8_e4m3
    )
    wh_np = np.concatenate([inputs["W_ha"], inputs["W_hv"]], axis=0).astype(
        ml_dtypes.bfloat16
    )
    bsum_np = (inputs["b_v"] + inputs["b_ha"] + inputs["b_hv"]).astype(np.float32)
    wf_np = inputs["W_f"][:, 0].astype(ml_dtypes.bfloat16)
    hs = np.concatenate([inputs["h_att"], inputs["prev_h2"]], axis=1)  # [B, 2*RNN]

    in_maps = []
    for i in range(NCORES):
        s = slice(i * BL, (i + 1) * BL)
        xtb_np, xt8_np = _prep_core(inputs["imgs_features"][s])
        in_maps.append(
            {
                "xt8": xt8_np,
                "xtb": xtb_np,
                "wv8": wv8_np,
                "wh": wh_np,
                "hsT": np.ascontiguousarray(hs[s].T).astype(ml_dtypes.bfloat16),
                "bsum": bsum_np,
                "wf": wf_np,
            }
        )

    trace = bool(os.environ.get("BASS_KERNEL_TRACE"))
    if trace:
        _install_ntff_shim()
    res = run_bass_kernel_spmd(nc, in_maps, list(range(NCORES)), trace=trace)
    if trace:
        _CACHE["last_results"] = res
        print(f"HW exec time: {res.exec_time_ns} ns")
    return np.concatenate([res.results[i]["out"] for i in range(NCORES)], axis=0)
